# revision 11
# baseline (speedup 1.0000x reference)
"""DCN cross-layer stack on 8 Trainium2 NeuronCores (data parallel over batch).

Math: with zero bias params the cross stack collapses to
    out[b, :] = x[b, :] * prod_i (1 + p_i[b]),   p_i = x @ W_i.
Everything runs in TRANSPOSED space with a bf16 wire format (the 2e-2
harness tolerance leaves ~4x margin at bf16):
    - host ships xT as a [128, 2, 1024] bf16 SBUF image,
    - PE computes P^T = W @ xT with W stationary (weight rows spread to
      psum partitions {0, 64, 32, 96}: the alpha product then runs as
      mixed PSUM/SBUF ops, the only cross-partition-base form the BIR
      verifier admits),
    - alpha = (1+p0)(1+p2) * (1+p1)(1+p3) via 3 DVE ops per chunk,
    - one ones-stationary matmul broadcasts alpha to all 128 partitions,
    - DVE/GpSimd multiply xT by the broadcast; outT returns bf16 and the
      host re-transposes / upcasts.
"""

import os
from contextlib import ExitStack

import ml_dtypes
import numpy as np

import concourse.bacc as bacc
import concourse.bass as bass
import concourse.tile as tile
from concourse.tile import add_dep_helper
from concourse import mybir
from concourse.bass_utils import run_bass_kernel_spmd

FP = mybir.dt.float32
BF = mybir.dt.bfloat16
BF_NP = ml_dtypes.bfloat16

B_FULL = 8192
D = 256
L = 4
N_CORES = 8
B_CORE = B_FULL // N_CORES  # 1024
NCH = 2                     # b chunks (psum bank limit: 512 fp32 per bank)
CW = B_CORE // NCH          # 512
# quad-spread psum rows for the weight columns: p0@0, p1@64, p2@32, p3@96
QROW = (0, 64, 32, 96)

_cache = {}
last_exec_time_ns = None
last_results = None


def _build_nc(qs, gamma_zero):
    """qs: tuple of L floats (q_i, parameter-only). gamma_zero: skip +gamma."""
    nc = bacc.Bacc(
        "TRN2", target_bir_lowering=False, debug=False, num_devices=N_CORES
    )
    xT_in = nc.declare_dram_parameter("xT", [128, 2, B_CORE], BF, isOutput=False)
    wq_in = nc.declare_dram_parameter("wq", [128, 2, 128], BF, isOutput=False)
    if not gamma_zero:
        gm_in = nc.declare_dram_parameter("gm", [128, 2], BF, isOutput=False)
    out_ext = nc.declare_dram_parameter("out", [128, 2, B_CORE], BF, isOutput=True)

    fast = gamma_zero and all(q == 0.0 for q in qs)
    AT = mybir.ActivationFunctionType
    OP = mybir.AluOpType

    with tile.TileContext(nc) as tc, ExitStack() as ctx:
        consts = ctx.enter_context(tc.tile_pool(name="consts", bufs=1))
        xin = ctx.enter_context(tc.tile_pool(name="xin", bufs=1))
        work = ctx.enter_context(tc.tile_pool(name="work", bufs=1))
        outp = ctx.enter_context(tc.tile_pool(name="outp", bufs=1))
        pps = ctx.enter_context(
            tc.tile_pool(name="pps", bufs=1, space=bass.MemorySpace.PSUM)
        )
        bps = ctx.enter_context(
            tc.tile_pool(name="bps", bufs=1, space=bass.MemorySpace.PSUM)
        )

        wq = consts.tile([128, 2, 128], BF)
        if not gamma_zero:
            gm = consts.tile([128, 2], BF)
            nc.scalar.dma_start(out=gm[:], in_=gm_in[:, :])
        ones = consts.tile([1, 128], BF)
        nc.vector.memset(ones[:], 1.0)

        # xT: one 256KB piece per chunk per HWDGE ring (both halves); the
        # tiny weight image leads the ACT ring so it lands before chunk0
        xt = xin.tile([128, 2, B_CORE], BF)
        cs = [slice(c * CW, (c + 1) * CW) for c in range(NCH)]
        nc.scalar.dma_start(out=wq[:], in_=wq_in[:, :, :])
        nc.sync.dma_start(out=xt[:, :, cs[0]], in_=xT_in[:, :, cs[0]])
        nc.scalar.dma_start(out=xt[:, :, cs[1]], in_=xT_in[:, :, cs[1]])

        # P^T per chunk: two accumulating matmuls, W stationary (quad layout)
        P = []
        for c in range(NCH):
            P_ps = pps.tile([128, CW], FP, tag=f"P{c}")
            nc.tensor.matmul(
                P_ps[:, :], wq[:, 0, :], xt[:, 0, cs[c]], start=True, stop=False
            )
            nc.tensor.matmul(
                P_ps[:, :], wq[:, 1, :], xt[:, 1, cs[c]], start=False, stop=True
            )
            P.append(P_ps)

        a1 = work.tile([65, B_CORE], BF, tag="a1")
        usb = work.tile([1, B_CORE], BF, tag="u")
        vsb = work.tile([1, B_CORE], BF, tag="v")
        alpha = work.tile([1, B_CORE], BF, tag="alpha")
        ab = work.tile([128, B_CORE], BF, tag="ab")
        ot = outp.tile([128, 2, B_CORE], BF, tag="ot")

        def chain(c):
            """alpha[cs[c]] from P[c]; returns the last DVE instruction."""
            nc.scalar.activation(a1[:, cs[c]], P[c][0:65, :], AT.Copy, bias=1.0)
            if fast:
                u_i = nc.vector.scalar_tensor_tensor(
                    usb[0:1, cs[c]], P[c][32:33, :], 1.0, a1[0:1, cs[c]],
                    op0=OP.add, op1=OP.mult,
                )
                nc.vector.scalar_tensor_tensor(
                    vsb[0:1, cs[c]], P[c][96:97, :], 1.0, a1[64:65, cs[c]],
                    op0=OP.add, op1=OP.mult,
                )
                al_i = nc.vector.tensor_mul(
                    alpha[0:1, cs[c]], usb[0:1, cs[c]], vsb[0:1, cs[c]]
                )
                return u_i, al_i
            # general recurrence a_{i+1} = a_i*(1+p_i) + q_i; q_0 is always 0
            # (gamma starts at zero), so a_1 = 1+p_0 = a1 row 0
            bufs = (usb, vsb, alpha)
            cur = a1[0:1, cs[c]]
            first = last = None
            for i in range(1, L):
                dst = bufs[i - 1][0:1, cs[c]]
                last = nc.vector.scalar_tensor_tensor(
                    dst, P[c][QROW[i]:QROW[i] + 1, :], 1.0, cur,
                    op0=OP.add, op1=OP.mult,
                )
                if first is None:
                    first = last
                if qs[i] != 0.0:
                    last = nc.vector.tensor_scalar_add(dst, dst, qs[i])
                cur = dst
            return first, last

        def bcast(c):
            """broadcast alpha chunk to psum + bf16 sbuf exit."""
            B_ps = bps.tile([128, CW], FP, tag=f"B{c}")
            nc.tensor.matmul(
                B_ps[:, :], ones[:, :], alpha[0:1, cs[c]], start=True, stop=True
            )
            nc.scalar.activation(ab[:, cs[c]], B_ps[:, :], AT.Copy)
            return B_ps

        u0_i, al0_i = chain(0)
        u1_i, al1_i = chain(1)
        # keep chunk0's alpha ahead of chunk1's chain on the DVE so B0 can
        # issue early (the scheduler otherwise batches both chains first)
        add_dep_helper(
            u1_i.ins, al0_i.ins,
            reason="finish chunk0 alpha before starting chunk1 chain",
        )
        B0 = bcast(0)
        B1 = bcast(1)
        # one fused multiply per chunk on the DVE at 2x bf16: ab broadcast
        # across the two d-halves via a stride-0 free dim (gpsimd stays idle
        # -- a concurrent gpsimd op knocks the DVE out of 2-port mode)
        for c in range(NCH):
            ab_b = ab[:, cs[c]].unsqueeze(1).broadcast_to((128, 2, CW))
            nc.vector.tensor_mul(ot[:, :, cs[c]], xt[:, :, cs[c]], ab_b)
        if not gamma_zero:
            for c in range(NCH):
                nc.vector.tensor_scalar_add(
                    ot[:, 0, cs[c]], ot[:, 0, cs[c]], gm[:, 0:1]
                )
                nc.vector.tensor_scalar_add(
                    ot[:, 1, cs[c]], ot[:, 1, cs[c]], gm[:, 1:2]
                )

        # outputs: chunk0 on the SP ring, chunk1 on the ACT ring
        nc.sync.dma_start(out=out_ext[:, :, cs[0]], in_=ot[:, :, cs[0]])
        nc.scalar.dma_start(out=out_ext[:, :, cs[1]], in_=ot[:, :, cs[1]])
    nc.finalize()
    return nc


def kernel(x, W, b_lin, bias):
    global last_exec_time_ns, last_results
    x = np.ascontiguousarray(x, dtype=np.float32)
    W = np.asarray(W, dtype=np.float32)
    b_lin = np.asarray(b_lin, dtype=np.float32)
    bias = np.asarray(bias, dtype=np.float32)

    # parameter-only precompute: gamma recurrence and q_i = gamma_i . W_i
    c = b_lin[:, None].astype(np.float64) + bias.astype(np.float64)  # [L, D]
    Wd = W.astype(np.float64)
    gamma = np.zeros(D, dtype=np.float64)
    q = np.zeros(L, dtype=np.float64)
    for i in range(L):
        q[i] = float(gamma @ Wd[i])
        gamma = gamma + c[i]
    gamma_zero = not np.any(gamma)
    q_f = tuple(float(np.float32(v)) for v in q)

    key = (q_f, gamma_zero)
    if key not in _cache:
        _cache[key] = _build_nc(q_f, gamma_zero)
    nc = _cache[key]

    # wq image: [p, h, col] with col QROW[l] = W[l, h*128+p], rest zero
    wq = np.zeros((128, 2, 128), dtype=BF_NP)
    Wb = W.astype(BF_NP)
    for l in range(L):
        for h in range(2):
            wq[:, h, QROW[l]] = Wb[l, h * 128:(h + 1) * 128]

    xb = x.astype(BF_NP)
    in_maps = []
    for core in range(N_CORES):
        xs = xb[core * B_CORE:(core + 1) * B_CORE]          # [1024, 256]
        xT = np.ascontiguousarray(
            xs.T.reshape(2, 128, B_CORE).transpose(1, 0, 2)  # [128, 2, 1024]
        )
        m = {"xT": xT, "wq": wq}
        if not gamma_zero:
            m["gm"] = np.ascontiguousarray(
                gamma.astype(BF_NP).reshape(2, 128).T
            )
        in_maps.append(m)

    trace = bool(os.environ.get("KERNEL_TRACE"))
    res = run_bass_kernel_spmd(nc, in_maps, list(range(N_CORES)), trace=trace)
    last_exec_time_ns = res.exec_time_ns
    last_results = res

    outs = []
    for core in range(N_CORES):
        o = np.asarray(res.results[core]["out"])             # [128, 2, 1024] bf16
        o = o.transpose(1, 0, 2).reshape(D, B_CORE).T        # [1024, 256]
        outs.append(o.astype(np.float32))
    return np.concatenate(outs, axis=0)


# revision 12
# speedup vs baseline: 1.0592x; 1.0592x over previous
"""DCN cross-layer stack on 8 Trainium2 NeuronCores (data parallel over batch).

Math: with zero bias params the cross stack collapses to
    out[b, :] = x[b, :] * prod_i (1 + p_i[b]),   p_i = x @ W_i.
Everything runs in TRANSPOSED space with a bf16 wire format (the 2e-2
harness tolerance leaves ~4x margin at bf16):
    - host ships xT as a [128, 2, 1024] bf16 SBUF image,
    - PE computes P^T = W @ xT with W stationary (weight rows spread to
      psum partitions {0, 64, 32, 96}: the alpha product then runs as
      mixed PSUM/SBUF ops, the only cross-partition-base form the BIR
      verifier admits),
    - alpha = (1+p0)(1+p2) * (1+p1)(1+p3) via 3 DVE ops per chunk,
    - one ones-stationary matmul broadcasts alpha to all 128 partitions,
    - DVE/GpSimd multiply xT by the broadcast; outT returns bf16 and the
      host re-transposes / upcasts.
"""

import os
from contextlib import ExitStack

import ml_dtypes
import numpy as np

import concourse.bacc as bacc
import concourse.bass as bass
import concourse.tile as tile
from concourse.tile import add_dep_helper
from concourse import mybir
from concourse.bass_utils import run_bass_kernel_spmd

FP = mybir.dt.float32
BF = mybir.dt.bfloat16
BF_NP = ml_dtypes.bfloat16

B_FULL = 8192
D = 256
L = 4
N_CORES = 8
B_CORE = B_FULL // N_CORES  # 1024
NCH = 2                     # b chunks (psum bank limit: 512 fp32 per bank)
CW = B_CORE // NCH          # 512
# quad-spread psum rows for the weight columns: p0@0, p1@64, p2@32, p3@96
QROW = (0, 64, 32, 96)

_cache = {}
last_exec_time_ns = None
last_results = None


def _build_nc(qs, gamma_zero):
    """qs: tuple of L floats (q_i, parameter-only). gamma_zero: skip +gamma."""
    nc = bacc.Bacc(
        "TRN2", target_bir_lowering=False, debug=False, num_devices=N_CORES
    )
    xT_in = nc.declare_dram_parameter("xT", [128, 2, B_CORE], BF, isOutput=False)
    wq_in = nc.declare_dram_parameter("wq", [128, 2, 128], BF, isOutput=False)
    if not gamma_zero:
        gm_in = nc.declare_dram_parameter("gm", [128, 2], BF, isOutput=False)
    out_ext = nc.declare_dram_parameter("out", [128, 2, B_CORE], BF, isOutput=True)

    fast = gamma_zero and all(q == 0.0 for q in qs)
    AT = mybir.ActivationFunctionType
    OP = mybir.AluOpType

    with tile.TileContext(nc) as tc, ExitStack() as ctx:
        consts = ctx.enter_context(tc.tile_pool(name="consts", bufs=1))
        xin = ctx.enter_context(tc.tile_pool(name="xin", bufs=1))
        work = ctx.enter_context(tc.tile_pool(name="work", bufs=1))
        outp = ctx.enter_context(tc.tile_pool(name="outp", bufs=1))
        pps = ctx.enter_context(
            tc.tile_pool(name="pps", bufs=1, space=bass.MemorySpace.PSUM)
        )
        bps = ctx.enter_context(
            tc.tile_pool(name="bps", bufs=1, space=bass.MemorySpace.PSUM)
        )

        wq = consts.tile([128, 2, 128], BF)
        if not gamma_zero:
            gm = consts.tile([128, 2], BF)
            nc.scalar.dma_start(out=gm[:], in_=gm_in[:, :])
        ones = consts.tile([1, 128], BF)
        nc.vector.memset(ones[:], 1.0)

        # xT: one 256KB piece per chunk per HWDGE ring (both halves); the
        # tiny weight image leads the ACT ring so it lands before chunk0
        xt = xin.tile([128, 2, B_CORE], BF)
        cs = [slice(c * CW, (c + 1) * CW) for c in range(NCH)]
        nc.scalar.dma_start(out=wq[:], in_=wq_in[:, :, :])
        nc.sync.dma_start(out=xt[:, :, cs[0]], in_=xT_in[:, :, cs[0]])
        nc.scalar.dma_start(out=xt[:, :, cs[1]], in_=xT_in[:, :, cs[1]])

        # P^T per chunk: two accumulating matmuls, W stationary (quad layout)
        P = []
        for c in range(NCH):
            P_ps = pps.tile([128, CW], FP, tag=f"P{c}")
            nc.tensor.matmul(
                P_ps[:, :], wq[:, 0, :], xt[:, 0, cs[c]], start=True, stop=False
            )
            nc.tensor.matmul(
                P_ps[:, :], wq[:, 1, :], xt[:, 1, cs[c]], start=False, stop=True
            )
            P.append(P_ps)

        a1 = work.tile([65, B_CORE], BF, tag="a1")
        usb = work.tile([1, B_CORE], BF, tag="u")
        vsb = work.tile([1, B_CORE], BF, tag="v")
        alpha = work.tile([1, B_CORE], BF, tag="alpha")
        ab = work.tile([128, B_CORE], BF, tag="ab")
        ot = outp.tile([128, 2, B_CORE], BF, tag="ot")

        def chain(c):
            """alpha[cs[c]] from P[c]; returns the last DVE instruction."""
            nc.scalar.activation(a1[:, cs[c]], P[c][0:65, :], AT.Copy, bias=1.0)
            if fast:
                u_i = nc.vector.scalar_tensor_tensor(
                    usb[0:1, cs[c]], P[c][32:33, :], 1.0, a1[0:1, cs[c]],
                    op0=OP.add, op1=OP.mult,
                )
                v_i = nc.vector.scalar_tensor_tensor(
                    vsb[0:1, cs[c]], P[c][96:97, :], 1.0, a1[64:65, cs[c]],
                    op0=OP.add, op1=OP.mult,
                )
                return u_i, v_i
            # general recurrence a_{i+1} = a_i*(1+p_i) + q_i; q_0 is always 0
            # (gamma starts at zero), so a_1 = 1+p_0 = a1 row 0
            bufs = (usb, vsb, alpha)
            cur = a1[0:1, cs[c]]
            first = last = None
            for i in range(1, L):
                dst = bufs[i - 1][0:1, cs[c]]
                last = nc.vector.scalar_tensor_tensor(
                    dst, P[c][QROW[i]:QROW[i] + 1, :], 1.0, cur,
                    op0=OP.add, op1=OP.mult,
                )
                if first is None:
                    first = last
                if qs[i] != 0.0:
                    last = nc.vector.tensor_scalar_add(dst, dst, qs[i])
                cur = dst
            return first, last

        def bcast(src_row, dst, tag):
            """broadcast a [1, CW] row to all partitions: psum + bf16 exit."""
            B_ps = bps.tile([128, CW], FP, tag=tag)
            nc.tensor.matmul(B_ps[:, :], ones[:, :], src_row, start=True, stop=True)
            nc.scalar.activation(dst, B_ps[:, :], AT.Copy)
            return B_ps

        u0_i, v0_i = chain(0)
        u1_i, v1_i = chain(1)
        # keep chunk0's chain ahead of chunk1's on the DVE (the scheduler
        # otherwise interleaves them and delays chunk0's broadcasts)
        add_dep_helper(
            u1_i.ins, v0_i.ins,
            reason="finish chunk0 u/v before starting chunk1 chain",
        )
        if fast:
            # alpha-free tail: broadcast u and v separately (PE is idle and
            # nothing waits on the u*v product), then two chained 2x bf16
            # multiplies per chunk; gpsimd stays idle (a concurrent gpsimd op
            # knocks the DVE out of 2-port mode)
            abv = work.tile([128, B_CORE], BF, tag="abv")
            tt = work.tile([128, 2, B_CORE], BF, tag="tt")
            for c in range(NCH):
                bcast(usb[0:1, cs[c]], ab[:, cs[c]], f"Bu{c}")
                bcast(vsb[0:1, cs[c]], abv[:, cs[c]], f"Bv{c}")
            for c in range(NCH):
                ab_b = ab[:, cs[c]].unsqueeze(1).broadcast_to((128, 2, CW))
                abv_b = abv[:, cs[c]].unsqueeze(1).broadcast_to((128, 2, CW))
                nc.vector.tensor_mul(tt[:, :, cs[c]], xt[:, :, cs[c]], ab_b)
                nc.vector.tensor_mul(ot[:, :, cs[c]], tt[:, :, cs[c]], abv_b)
        else:
            for c in range(NCH):
                bcast(alpha[0:1, cs[c]], ab[:, cs[c]], f"B{c}")
            for c in range(NCH):
                ab_b = ab[:, cs[c]].unsqueeze(1).broadcast_to((128, 2, CW))
                nc.vector.tensor_mul(ot[:, :, cs[c]], xt[:, :, cs[c]], ab_b)
            for c in range(NCH):
                nc.vector.tensor_scalar_add(
                    ot[:, 0, cs[c]], ot[:, 0, cs[c]], gm[:, 0:1]
                )
                nc.vector.tensor_scalar_add(
                    ot[:, 1, cs[c]], ot[:, 1, cs[c]], gm[:, 1:2]
                )

        # outputs: chunk0 on the SP ring, chunk1 on the ACT ring
        nc.sync.dma_start(out=out_ext[:, :, cs[0]], in_=ot[:, :, cs[0]])
        nc.scalar.dma_start(out=out_ext[:, :, cs[1]], in_=ot[:, :, cs[1]])
    nc.finalize()
    return nc


def kernel(x, W, b_lin, bias):
    global last_exec_time_ns, last_results
    x = np.ascontiguousarray(x, dtype=np.float32)
    W = np.asarray(W, dtype=np.float32)
    b_lin = np.asarray(b_lin, dtype=np.float32)
    bias = np.asarray(bias, dtype=np.float32)

    # parameter-only precompute: gamma recurrence and q_i = gamma_i . W_i
    c = b_lin[:, None].astype(np.float64) + bias.astype(np.float64)  # [L, D]
    Wd = W.astype(np.float64)
    gamma = np.zeros(D, dtype=np.float64)
    q = np.zeros(L, dtype=np.float64)
    for i in range(L):
        q[i] = float(gamma @ Wd[i])
        gamma = gamma + c[i]
    gamma_zero = not np.any(gamma)
    q_f = tuple(float(np.float32(v)) for v in q)

    key = (q_f, gamma_zero)
    if key not in _cache:
        _cache[key] = _build_nc(q_f, gamma_zero)
    nc = _cache[key]

    # wq image: [p, h, col] with col QROW[l] = W[l, h*128+p], rest zero
    wq = np.zeros((128, 2, 128), dtype=BF_NP)
    Wb = W.astype(BF_NP)
    for l in range(L):
        for h in range(2):
            wq[:, h, QROW[l]] = Wb[l, h * 128:(h + 1) * 128]

    xb = x.astype(BF_NP)
    in_maps = []
    for core in range(N_CORES):
        xs = xb[core * B_CORE:(core + 1) * B_CORE]          # [1024, 256]
        xT = np.ascontiguousarray(
            xs.T.reshape(2, 128, B_CORE).transpose(1, 0, 2)  # [128, 2, 1024]
        )
        m = {"xT": xT, "wq": wq}
        if not gamma_zero:
            m["gm"] = np.ascontiguousarray(
                gamma.astype(BF_NP).reshape(2, 128).T
            )
        in_maps.append(m)

    trace = bool(os.environ.get("KERNEL_TRACE"))
    res = run_bass_kernel_spmd(nc, in_maps, list(range(N_CORES)), trace=trace)
    last_exec_time_ns = res.exec_time_ns
    last_results = res

    outs = []
    for core in range(N_CORES):
        o = np.asarray(res.results[core]["out"])             # [128, 2, 1024] bf16
        o = o.transpose(1, 0, 2).reshape(D, B_CORE).T        # [1024, 256]
        outs.append(o.astype(np.float32))
    return np.concatenate(outs, axis=0)
